# revision 38
# baseline (speedup 1.0000x reference)
"""Self-contained TRN2 Bass kernel for nn_CAM_Module (channel attention).

kernel(x, gamma): x [16,512,64,64] f32, gamma [1] f32 -> [16,512,64,64] f32.
Data-parallel over batch: 2 samples per NeuronCore across 8 cores.

Math: q = x.reshape(B,C,HW); E = q@q.T; softmax(rowmax(E)-E) == softmax(-E)
(shift invariance). Key folds:
  out = gamma*softmax(-E)@q + x = (gamma*softmax(-E) + I) @ q   since x == q
  gamma/Z scaling folded into the exp bias: A' = exp(rowmin + ln(gamma)
  - ln(Z) - E); M = A' + I; out = M @ q directly in PSUM -> epilogue is a
  plain PSUM->SBUF copy (split DVE/ACT) instead of scalar_tensor_tensor.

On-chip strategy (per core, 2 samples):
  - load fp32 in progressive pieces, cast to fp16 (DVE/ACT), PE-transpose
    128x128 tiles to build q^T chunks; single-pass fp16 Gram accumulated in
    fp32 PSUM (upper-triangle blocks only, packed into 3 PSUM banks),
    mirrored via fp16 PE transposes of the fp16 E_sb copy.
  - softmax: exp pass1 (fused rowsum via accum_out) -> ln(Z) on ACT ->
    exp pass2 with bias = rowmin + ln(gamma) - ln(Z) -> diag +1 add.
  - A-matmul fp16 with M^T tiles; PSUM holds the final output; evacuation
    copies alternate DVE/ACT into 1MB staging tiles.
  - PE pre-warmed with dummy transposes during the load ramp (HAM);
    softmax/mirror/expT steps interleave with Gram/A-phase emission so the
    PE never idles long enough to re-throttle.
"""
import sys
if '/opt/trn_rl_repo' not in sys.path:
    sys.path.insert(0, '/opt/trn_rl_repo')
import numpy as np
import concourse.bass as bass
import concourse.tile as tile
import concourse.mybir as mybir
from concourse.masks import make_identity

F32 = mybir.dt.float32
F16 = mybir.dt.float16
F8 = mybir.dt.float8e4

C = 512          # channels
N = 4096         # spatial (64*64)
CB = C // 128    # 4 c-blocks
NK = N // 128    # 32 transpose chunks
NG = NK // 2     # 16 transpose groups (2 chunks per PSUM bounce bank)
NO = N // 512    # 8 output column chunks
S = 2            # samples per core
WARM = 48        # dummy transposes to pre-warm the PE HAM clock gate

# load piece widths (columns) and offsets: finer first pieces cut the
# head latency; 512KB steady-state pieces pipeline the Gram phase
# against DMA arrival without starving the PE
P_W = [512, 512, 1024, 1024, 1024]
P_OFF = [0, 512, 1024, 2048, 3072]
NP = len(P_W)
# packed E PSUM layout: row-block m -> (offset, width); fits 3 banks,
# no block crosses a 2KB bank boundary (m3 placed before m2)
EW = [512, 384, 256, 128]
EOFF = [0, 512, 1024, 896]  # m0@0 m1@512 m3@896 m2@1024
MIRROR_IJ = [(1, 0), (2, 0), (2, 1), (3, 0), (3, 1), (3, 2)]


def _piece_of(n0):
    for p in range(NP):
        if P_OFF[p] <= n0 < P_OFF[p] + P_W[p]:
            return p, n0 - P_OFF[p]
    raise AssertionError(n0)


def build(nc: bass.Bass):
    x_ext = nc.declare_dram_parameter("x", [S * C, N], F32, isOutput=False)
    g_ext = nc.declare_dram_parameter("gamma", [1, 1], F32, isOutput=False)
    out_ext = nc.declare_dram_parameter("out", [S * C, N], F32, isOutput=True)
    x_ap = x_ext.ap()
    out_ap = out_ext.ap()

    with tile.TileContext(nc) as tc:
        with (
            tc.tile_pool(name="const", bufs=1) as const,
            tc.tile_pool(name="x32", bufs=2) as x32,
            tc.tile_pool(name="q16", bufs=S * CB) as q16p,
            tc.tile_pool(name="qt", bufs=6) as qtp,
            tc.tile_pool(name="esb", bufs=2) as esbp,
            tc.tile_pool(name="expn", bufs=2) as expnp,
            tc.tile_pool(name="expt", bufs=2 * 2) as exptp,
            tc.tile_pool(name="q8", bufs=2 * 2) as q8p,
            tc.tile_pool(name="vecs", bufs=8) as vecs,
            tc.tile_pool(name="outs", bufs=3) as outsp,
            tc.tile_pool(name="ps_bounce", bufs=2, space="PSUM") as ps_t,
            tc.tile_pool(name="ps_e", bufs=1, space="PSUM") as ps_e,
            tc.tile_pool(name="ps_o", bufs=3, space="PSUM") as ps_o,
        ):
            # PE pre-warm: back-to-back dummy matmuls on a DVE-memset
            # scratch flip the HAM clock gate to 8/8 before the real
            # stream begins -- no dependency on the (slow, gpsimd-built)
            # identity, so the PE starts almost immediately
            scratch = const.tile([128, 128], F16)
            nc.vector.memset(scratch, 0.0)
            warm = ps_t.tile([128, 512], F32, tag="bounce", name="warm")
            for _ in range(WARM):
                nc.tensor.matmul(warm[:, 0:128], lhsT=scratch[:],
                                 rhs=scratch[:], start=True, stop=True)

            ident = const.tile([128, 128], F16)
            make_identity(nc, ident)
            # anti-diagonal J: transpose(x, J) = x^T with columns
            # reversed -- builds the DoubleRowSwInterleave weight layout
            # (descending logical column order) for free on the PE
            antij = const.tile([128, 128], F16)
            nc.gpsimd.memset(antij, 0.0)
            nc.gpsimd.affine_select(
                out=antij,
                in_=antij,
                compare_op=mybir.AluOpType.not_equal,
                fill=1.0,
                base=-127,
                pattern=[[1, 128]],
                channel_multiplier=1,
            )
            gbc = const.tile([128, 1], F32)
            nc.gpsimd.dma_start(out=gbc, in_=g_ext.ap().to_broadcast((128, 1)))
            lng = const.tile([128, 1], F32)
            nc.scalar.activation(lng, gbc, mybir.ActivationFunctionType.Ln)

            st = [dict() for _ in range(S)]

            def warm_fill(n):
                # keep the PE stream dense during the load ramp (HAM)
                for _ in range(n):
                    nc.tensor.matmul(warm[:, 0:128], lhsT=scratch[:],
                                     rhs=scratch[:], start=True, stop=True)

            def load_piece(s, p):
                # casts all on DVE (2x mode for fp32-src copies; ACT gets
                # no accel on fp32 sources and was the v2 bottleneck)
                if "q16" not in st[s]:
                    st[s]["q16"] = [[None] * NP for _ in range(CB)]
                    st[s]["qtc"] = {}
                q16 = st[s]["q16"]
                for cb in range(CB):
                    xt = x32.tile([128, P_W[p]], F32, tag=f"xt{p}",
                                  name=f"xt_{s}_{cb}_{p}")
                    nc.sync.dma_start(
                        out=xt,
                        in_=x_ap[
                            s * C + cb * 128 : s * C + (cb + 1) * 128,
                            P_OFF[p] : P_OFF[p] + P_W[p],
                        ],
                    )
                    qc = q16p.tile([128, P_W[p]], F16, tag=f"q16_{p}",
                                   name=f"q16_{s}_{cb}_{p}")
                    nc.vector.tensor_copy(qc[:], xt[:])
                    q16[cb][p] = qc

            def tgroup(s, g):
                # transpose 2 chunks (8 [128,128] fp16 tiles) into one
                # PSUM bounce bank, evacuate to SBUF in one op
                q16 = st[s]["q16"]
                bounce = ps_t.tile([128, 2, CB, 128], F16, tag="bounce",
                                   name=f"bounce_{s}_{g}")
                for h in range(2):
                    k = 2 * g + h
                    kp, ko = _piece_of(128 * k)
                    for cb in range(CB):
                        nc.tensor.transpose(
                            bounce[:, h, cb, :],
                            q16[cb][kp][:, ko : ko + 128],
                            ident,
                        )
                qtc = qtp.tile([128, 2, CB * 128], F16, tag="qtc",
                               name=f"qtc_{s}_{g}")
                # DVE reads fp16 PSUM at 2x (650ns), ACT only 1x (1.1us);
                # alternate so neither queue gates the bounce-bank ring
                if g % 2 == 0:
                    nc.vector.tensor_copy(qtc[:], bounce[:, :, :, :])
                else:
                    nc.scalar.copy(qtc[:], bounce[:, :, :, :])
                st[s]["qtc"][g] = qtc

            def emm(s, g):
                # symmetric Gram accumulation: upper-triangle blocks only,
                # packed E layout (3 PSUM banks)
                if "E" not in st[s]:
                    st[s]["E"] = ps_e.tile([128, 1280], F32, tag="E",
                                           name=f"E_{s}")
                E = st[s]["E"]
                qtc = st[s]["qtc"].pop(g)
                for h in range(2):
                    k = 2 * g + h
                    for m in range(CB):
                        # m=1 and m=3 share a PSUM bank; start=True clears
                        # has_written BANK-wide, so only m=1 may start the
                        # bank — m=3's first write lands on cleared bits
                        # (overwrite semantics) right after m=1's start
                        nc.tensor.matmul(
                            E[:, EOFF[m] : EOFF[m] + EW[m]],
                            lhsT=qtc[:, h, m * 128 : (m + 1) * 128],
                            rhs=qtc[:, h, m * 128 : 512],
                            start=(k == 0 and m != 3),
                            stop=(k == NK - 1),
                        )

            def esb_evac(s):
                # rebuild upper-tri rows in SBUF as fp16 (frees E PSUM)
                E = st[s]["E"]
                E_sb = esbp.tile([128, CB, 512], F16, tag="esb",
                                 name=f"esb_{s}")
                for m in range(CB):
                    nc.scalar.copy(E_sb[:, m, m * 128 : 512],
                                   E[:, EOFF[m] : EOFF[m] + EW[m]])
                st[s]["E_sb"] = E_sb

            def mirror_step(s, idx):
                # mirror one lower-triangle block via a fp16 PE transpose
                i, j = MIRROR_IJ[idx]
                E_sb = st[s]["E_sb"]
                tb = ps_o.tile([128, 128], F16, tag="acc",
                               name=f"tb_{s}_{i}_{j}")
                nc.tensor.transpose(
                    tb[:], E_sb[:, j, i * 128 : (i + 1) * 128], ident
                )
                nc.scalar.copy(E_sb[:, i, j * 128 : (j + 1) * 128], tb[:])

            def softmax_m(s, m):
                # A' = gamma/Z * exp(rowmin - E) via double exp pass;
                # then M = A' + I (diag add of the fp16 identity)
                E_sb = st[s]["E_sb"]
                if "expn" not in st[s]:
                    st[s]["expn"] = expnp.tile([128, CB, 512], F16,
                                               tag="expn", name=f"expn_{s}")
                expn = st[s]["expn"]
                mv = vecs.tile([128, 1], F32, tag="mv", name=f"mv_{s}_{m}")
                nc.vector.tensor_reduce(
                    mv, E_sb[:, m, :], axis=mybir.AxisListType.X,
                    op=mybir.AluOpType.min,
                )
                Z = vecs.tile([128, 1], F32, tag="Z", name=f"Z_{s}_{m}")
                nc.scalar.activation(
                    expn[:, m, :],
                    E_sb[:, m, :],
                    mybir.ActivationFunctionType.Exp,
                    bias=mv,
                    scale=-1.0,
                    accum_out=Z,
                )
                lnZ = vecs.tile([128, 1], F32, tag="lnZ", name=f"lnZ_{s}_{m}")
                nc.scalar.activation(lnZ, Z, mybir.ActivationFunctionType.Ln)
                mvg = vecs.tile([128, 1], F32, tag="mvg", name=f"mvg_{s}_{m}")
                nc.vector.tensor_add(mvg, mv, lng)
                b2 = vecs.tile([128, 1], F32, tag="b2", name=f"b2_{s}_{m}")
                nc.vector.tensor_sub(b2, mvg, lnZ)
                nc.scalar.activation(
                    expn[:, m, :],
                    E_sb[:, m, :],
                    mybir.ActivationFunctionType.Exp,
                    bias=b2,
                    scale=-1.0,
                )
                # residual identity is added via a separate fp16 matmul in
                # the A-phase (keeps x at fp16 precision on the fp8 path)

            def q8_cast(s, pair):
                # fp8 copy of q for the DoubleRow rhs: q8[k, i, n] =
                # q[pair*256 + i*128 + k, n]; casts split DVE/ACT
                q16 = st[s]["q16"]
                q8 = q8p.tile([128, 2, N], F8, tag="q8",
                              name=f"q8_{s}_{pair}")
                for i in range(2):
                    for p in range(NP):
                        src = q16[2 * pair + i][p]
                        dst = q8[:, i, P_OFF[p] : P_OFF[p] + P_W[p]]
                        if (i + p) % 2 == 0:
                            nc.vector.tensor_copy(dst, src[:])
                        else:
                            nc.scalar.copy(dst, src[:])
                st[s].setdefault("q8", {})[pair] = q8

            def expT8_step(s, pair):
                # SwInterleave fp8 weights for d-pair `pair`:
                # raw[k, cb, c'', i] = A'^T[pair*256+i*128+k,
                # cb*128 + 127-c''] built via J-transposes + one
                # interleaving fp8 evacuation copy
                expn = st[s]["expn"]
                bounce = ps_t.tile([128, 2, CB, 128], F16, tag="bounce",
                                   name=f"ebounce_{s}_{pair}")
                for i in range(2):
                    d = 2 * pair + i
                    for cb in range(CB):
                        nc.tensor.transpose(
                            bounce[:, i, cb, :],
                            expn[:, cb, d * 128 : (d + 1) * 128],
                            antij,
                        )
                raw = exptp.tile([128, CB, 128, 2], F8, tag="expT8",
                                 name=f"expT8_{s}_{pair}")
                src = bounce[:, :, :, :].rearrange("k i cb c -> k cb c i")
                if pair % 2 == 0:
                    nc.scalar.copy(raw[:], src)
                else:
                    nc.vector.tensor_copy(raw[:], src)
                st[s].setdefault("expT8", {})[pair] = raw

            def aphase_cb(s, cb, ostage):
                # out = A'@q8 (two fp8 DoubleRowSwInterleave matmuls over
                # d-pairs) + I@q16 (fp16 residual), in no-PAIRS sharing
                # each weight load (2nd matmul sets ldweights=False); 3
                # acc banks keep the pair pipeline free of evac stalls.
                # PSUM holds the final values; evac copies split DVE/ACT
                q16, q8 = st[s]["q16"], st[s]["q8"]
                expT8 = st[s]["expT8"]
                accs = {}
                for pair in range(NO // 2):
                    nos = (2 * pair, 2 * pair + 1)
                    for no in nos:
                        accs[no] = ps_o.tile([128, 512], F32, tag="acc",
                                             name=f"acc_{s}_{no}_{cb}")
                    for dp in range(2):
                        for idx, no in enumerate(nos):
                            nof = no * 512
                            mm = nc.tensor.matmul(
                                accs[no][:],
                                lhsT=expT8[dp][:, cb, :, :],
                                rhs=q8[dp][:, :, nof : nof + 512],
                                perf_mode=(
                                    mybir.MatmulPerfMode.DoubleRowSwInterleave
                                ),
                                start=(dp == 0),
                                stop=False,
                            )
                            if idx == 1:
                                mm.ldweights = False
                    for idx, no in enumerate(nos):
                        npc, nof = _piece_of(no * 512)
                        mm = nc.tensor.matmul(
                            accs[no][:],
                            lhsT=ident[:],
                            rhs=q16[cb][npc][:, nof : nof + 512],
                            start=False,
                            stop=True,
                        )
                        if idx == 1:
                            mm.ldweights = False
                    for idx, no in enumerate(nos):
                        half = no // (NO // 2)
                        if (cb, half) not in ostage:
                            ot = outsp.tile([128, (NO // 2) * 512], F32,
                                            tag="ot",
                                            name=f"ot_{s}_{cb}_{half}")
                            ostage[(cb, half)] = ot
                        ot = ostage[(cb, half)]
                        osl = slice((no % (NO // 2)) * 512,
                                    (no % (NO // 2) + 1) * 512)
                        if idx == 0:
                            nc.vector.tensor_copy(ot[:, osl], accs[no][:])
                        else:
                            nc.scalar.copy(ot[:, osl], accs[no][:])
                        _store(s, cb, no, ot)

            def _store(s, cb, no, ot):
                # the very last c-block stores in 512KB quarters so the
                # final DMA starts right after the last evac copy
                fine = (s == S - 1 and cb == CB - 1)
                if fine and no % 2 == 1:
                    q0 = (no - 1) % (NO // 2)
                    nc.sync.dma_start(
                        out=out_ap[
                            s * C + cb * 128 : s * C + (cb + 1) * 128,
                            (no - 1) * 512 : (no + 1) * 512,
                        ],
                        in_=ot[:, q0 * 512 : (q0 + 2) * 512],
                    )
                elif not fine and no % (NO // 2) == NO // 2 - 1:
                    half = no // (NO // 2)
                    nc.sync.dma_start(
                        out=out_ap[
                            s * C + cb * 128 : s * C + (cb + 1) * 128,
                            half * (NO // 2) * 512 :
                            (half + 1) * (NO // 2) * 512,
                        ],
                        in_=ot[:],
                    )

            # ---- interleaved emission schedule -----------------------
            # sample-0 loads up front, sample-1 load pieces interleaved
            # into the Gram(0) loop so DMA issue order matches arrival.
            # tgroups run TWO ahead of their emms: the bounce evacuation
            # of group g overlaps group g+1's transposes, so emm(g) never
            # exposes the evac latency on the PE queue
            for p in range(NP):
                load_piece(0, p)
            tgroup(0, 0)
            warm_fill(12)
            tgroup(0, 1)
            warm_fill(12)
            emm(0, 0)
            emm(0, 1)
            for g in range(2, NG):
                if g in (4, 8, 12):
                    # small transpose fills bridge DMA-arrival hiccups at
                    # load-piece boundaries so the HAM stays warm
                    warm_fill(4)
                tgroup(0, g)
                emm(0, g)
                if g == 8:
                    load_piece(1, 0)
                elif g == 10:
                    load_piece(1, 1)
                elif g == 12:
                    load_piece(1, 2)
                elif g == 14:
                    load_piece(1, 3)
            load_piece(1, 4)
            # E(0) PSUM is freed by esb_evac (all-ACT) so emm(1) can
            # start right after the evacuation copies
            esb_evac(0)
            tgroup(1, 0)
            tgroup(1, 1)
            emm(1, 0)
            mirror_step(0, 0)
            tgroup(1, 2); emm(1, 1)
            mirror_step(0, 1)
            tgroup(1, 3); emm(1, 2)
            mirror_step(0, 2)
            tgroup(1, 4); emm(1, 3)
            mirror_step(0, 3)
            tgroup(1, 5); emm(1, 4)
            mirror_step(0, 4)
            tgroup(1, 6); emm(1, 5)
            mirror_step(0, 5)
            for g in range(7, 11):
                tgroup(1, g)
                emm(1, g - 1)
            # softmax(0) in two chunks, early enough that the exp chain
            # completes before the PE reaches expT8(0), late enough that
            # the remaining qtc evacuations stay ahead of it; q8(0)
            # casts ride along on whatever engine slack remains
            softmax_m(0, 0); softmax_m(0, 1)
            q8_cast(0, 0)
            for g in range(11, 14):
                tgroup(1, g)
                emm(1, g - 1)
            softmax_m(0, 2); softmax_m(0, 3)
            q8_cast(0, 1)
            tgroup(1, 14); emm(1, 13)
            tgroup(1, 15); emm(1, 14)
            emm(1, 15)
            expT8_step(0, 0)
            expT8_step(0, 1)
            # aphase(0) with softmax(1) steps interleaved between c-blocks
            ostage0 = {}
            esb_evac(1)
            aphase_cb(0, 0, ostage0)
            mirror_step(1, 0); mirror_step(1, 1); mirror_step(1, 2)
            aphase_cb(0, 1, ostage0)
            mirror_step(1, 3); mirror_step(1, 4); mirror_step(1, 5)
            softmax_m(1, 0)
            q8_cast(1, 0)
            aphase_cb(0, 2, ostage0)
            softmax_m(1, 1); softmax_m(1, 2); softmax_m(1, 3)
            q8_cast(1, 1)
            aphase_cb(0, 3, ostage0)
            expT8_step(1, 0)
            expT8_step(1, 1)
            ostage1 = {}
            for cb in range(CB):
                aphase_cb(1, cb, ostage1)
    return nc


def _split_excess_waits(nc, max_waits=1):
    """This container's walrus rejects >1 sync-wait on one instruction
    ("Too many sync wait commands"); hoist extras onto standalone
    InstEventSemaphore preludes on the same engine."""
    n = 0
    for fn in nc.m.functions:
        for bb in fn.blocks:
            out = []
            for inst in bb.instructions:
                si = inst.sync_info
                if si is not None and si.on_wait and len(si.on_wait) > max_waits:
                    waits = list(si.on_wait)
                    head, keep = waits[:-max_waits], waits[-max_waits:]
                    for i, w in enumerate(head):
                        ev = mybir.InstEventSemaphore(
                            name=f"{inst.name}-wsplit{i}", ins=[], outs=[])
                        ev.engine = inst.engine
                        ev.sync_info = mybir.SyncInfo(on_wait=[w], on_update=[])
                        out.append(ev)
                        n += 1
                    inst.sync_info = mybir.SyncInfo(
                        on_wait=keep, on_update=list(si.on_update))
                out.append(inst)
            bb.instructions[:] = out
    return n


_cache = {}


def _get_nc():
    if 'nc' not in _cache:
        nc = bass.Bass()
        build(nc)
        _split_excess_waits(nc)
        _cache['nc'] = nc
    return _cache['nc']


def kernel(x: np.ndarray, gamma: np.ndarray) -> np.ndarray:
    from concourse.bass_utils import run_bass_kernel_spmd

    B, CH, H, W = x.shape          # (16, 512, 64, 64)
    NSP = H * W
    M = 8                          # cores
    SS = B // M                    # samples per core
    nc = _get_nc()
    g = np.ascontiguousarray(gamma, dtype=np.float32).reshape(1, 1)
    in_maps = [
        {
            "x": np.ascontiguousarray(
                x[i * SS : (i + 1) * SS].reshape(SS * CH, NSP), dtype=np.float32
            ),
            "gamma": g,
        }
        for i in range(M)
    ]
    res = run_bass_kernel_spmd(nc, in_maps, core_ids=list(range(M)))
    out = np.concatenate(
        [res.results[i]["out"].reshape(SS, CH, H, W) for i in range(M)], axis=0
    )
    return np.ascontiguousarray(out, dtype=np.float32)


# revision 41
# speedup vs baseline: 1.0293x; 1.0293x over previous
"""Self-contained TRN2 Bass kernel for nn_CAM_Module (channel attention).

kernel(x, gamma): x [16,512,64,64] f32, gamma [1] f32 -> [16,512,64,64] f32.
Data-parallel over batch: 2 samples per NeuronCore across 8 cores.

Math: q = x.reshape(B,C,HW); E = q@q.T; softmax(rowmax(E)-E) == softmax(-E)
(shift invariance). Key folds:
  out = gamma*softmax(-E)@q + x = (gamma*softmax(-E) + I) @ q   since x == q
  gamma/Z scaling folded into the exp bias: A' = exp(rowmin + ln(gamma)
  - ln(Z) - E); M = A' + I; out = M @ q directly in PSUM -> epilogue is a
  plain PSUM->SBUF copy (split DVE/ACT) instead of scalar_tensor_tensor.

On-chip strategy (per core, 2 samples):
  - load fp32 in progressive pieces, cast to fp16 (DVE/ACT), PE-transpose
    128x128 tiles to build q^T chunks; single-pass fp16 Gram accumulated in
    fp32 PSUM (upper-triangle blocks only, packed into 3 PSUM banks),
    mirrored via fp16 PE transposes of the fp16 E_sb copy.
  - softmax: exp pass1 (fused rowsum via accum_out) -> ln(Z) on ACT ->
    exp pass2 with bias = rowmin + ln(gamma) - ln(Z) -> diag +1 add.
  - A-matmul fp16 with M^T tiles; PSUM holds the final output; evacuation
    copies alternate DVE/ACT into 1MB staging tiles.
  - PE pre-warmed with dummy transposes during the load ramp (HAM);
    softmax/mirror/expT steps interleave with Gram/A-phase emission so the
    PE never idles long enough to re-throttle.
"""
import sys
if '/opt/trn_rl_repo' not in sys.path:
    sys.path.insert(0, '/opt/trn_rl_repo')
import numpy as np
import concourse.bass as bass
import concourse.tile as tile
import concourse.mybir as mybir
from concourse.masks import make_identity

F32 = mybir.dt.float32
F16 = mybir.dt.float16

C = 512          # channels
N = 4096         # spatial (64*64)
CB = C // 128    # 4 c-blocks
NK = N // 128    # 32 transpose chunks
NG = NK // 2     # 16 transpose groups (2 chunks per PSUM bounce bank)
NO = N // 512    # 8 output column chunks
S = 2            # samples per core
WARM = 48        # dummy transposes to pre-warm the PE HAM clock gate

# load piece widths (columns) and offsets: finer first pieces cut the
# head latency; 512KB steady-state pieces pipeline the Gram phase
# against DMA arrival without starving the PE
P_W = [512, 512, 1024, 1024, 1024]
P_OFF = [0, 512, 1024, 2048, 3072]
NP = len(P_W)
# packed E PSUM layout: row-block m -> (offset, width); fits 3 banks,
# no block crosses a 2KB bank boundary (m3 placed before m2)
EW = [512, 384, 256, 128]
EOFF = [0, 512, 1024, 896]  # m0@0 m1@512 m3@896 m2@1024
MIRROR_IJ = [(1, 0), (2, 0), (2, 1), (3, 0), (3, 1), (3, 2)]


def _piece_of(n0):
    for p in range(NP):
        if P_OFF[p] <= n0 < P_OFF[p] + P_W[p]:
            return p, n0 - P_OFF[p]
    raise AssertionError(n0)


def build(nc: bass.Bass):
    x_ext = nc.declare_dram_parameter("x", [S * C, N], F32, isOutput=False)
    g_ext = nc.declare_dram_parameter("gamma", [1, 1], F32, isOutput=False)
    out_ext = nc.declare_dram_parameter("out", [S * C, N], F32, isOutput=True)
    x_ap = x_ext.ap()
    out_ap = out_ext.ap()

    with tile.TileContext(nc) as tc:
        with (
            tc.tile_pool(name="const", bufs=1) as const,
            tc.tile_pool(name="x32", bufs=3) as x32,
            tc.tile_pool(name="q16", bufs=S * CB) as q16p,
            tc.tile_pool(name="qt", bufs=6) as qtp,
            tc.tile_pool(name="esb", bufs=2) as esbp,
            tc.tile_pool(name="expn", bufs=2) as expnp,
            tc.tile_pool(name="expt", bufs=2 * CB) as exptp,
            tc.tile_pool(name="vecs", bufs=8) as vecs,
            tc.tile_pool(name="outs", bufs=3) as outsp,
            tc.tile_pool(name="ps_bounce", bufs=2, space="PSUM") as ps_t,
            tc.tile_pool(name="ps_e", bufs=1, space="PSUM") as ps_e,
            tc.tile_pool(name="ps_o", bufs=3, space="PSUM") as ps_o,
        ):
            # PE pre-warm: back-to-back dummy matmuls on a DVE-memset
            # scratch flip the HAM clock gate to 8/8 before the real
            # stream begins -- no dependency on the (slow, gpsimd-built)
            # identity, so the PE starts almost immediately
            scratch = const.tile([128, 128], F16)
            nc.vector.memset(scratch, 0.0)
            warm = ps_t.tile([128, 512], F32, tag="bounce", name="warm")
            for _ in range(WARM):
                nc.tensor.matmul(warm[:, 0:128], lhsT=scratch[:],
                                 rhs=scratch[:], start=True, stop=True)

            ident = const.tile([128, 128], F16)
            make_identity(nc, ident)
            gbc = const.tile([128, 1], F32)
            nc.gpsimd.dma_start(out=gbc, in_=g_ext.ap().to_broadcast((128, 1)))
            lng = const.tile([128, 1], F32)
            nc.scalar.activation(lng, gbc, mybir.ActivationFunctionType.Ln)

            st = [dict() for _ in range(S)]

            def warm_fill(n):
                # keep the PE stream dense during the load ramp (HAM)
                for _ in range(n):
                    nc.tensor.matmul(warm[:, 0:128], lhsT=scratch[:],
                                     rhs=scratch[:], start=True, stop=True)

            def load_piece(s, p):
                # casts all on DVE (2x mode for fp32-src copies; ACT gets
                # no accel on fp32 sources and was the v2 bottleneck)
                if "q16" not in st[s]:
                    st[s]["q16"] = [[None] * NP for _ in range(CB)]
                    st[s]["qtc"] = {}
                q16 = st[s]["q16"]
                for cb in range(CB):
                    xt = x32.tile([128, P_W[p]], F32, tag=f"xt{p}",
                                  name=f"xt_{s}_{cb}_{p}")
                    nc.sync.dma_start(
                        out=xt,
                        in_=x_ap[
                            s * C + cb * 128 : s * C + (cb + 1) * 128,
                            P_OFF[p] : P_OFF[p] + P_W[p],
                        ],
                    )
                    qc = q16p.tile([128, P_W[p]], F16, tag=f"q16_{p}",
                                   name=f"q16_{s}_{cb}_{p}")
                    nc.vector.tensor_copy(qc[:], xt[:])
                    q16[cb][p] = qc

            def tgroup(s, g):
                # transpose 2 chunks (8 [128,128] fp16 tiles) into one
                # PSUM bounce bank, evacuate to SBUF in one op
                q16 = st[s]["q16"]
                bounce = ps_t.tile([128, 2, CB, 128], F16, tag="bounce",
                                   name=f"bounce_{s}_{g}")
                for h in range(2):
                    k = 2 * g + h
                    kp, ko = _piece_of(128 * k)
                    for cb in range(CB):
                        nc.tensor.transpose(
                            bounce[:, h, cb, :],
                            q16[cb][kp][:, ko : ko + 128],
                            ident,
                        )
                qtc = qtp.tile([128, 2, CB * 128], F16, tag="qtc",
                               name=f"qtc_{s}_{g}")
                # DVE reads fp16 PSUM at 2x (650ns), ACT only 1x (1.1us);
                # alternate so neither queue gates the bounce-bank ring
                if g % 2 == 0:
                    nc.vector.tensor_copy(qtc[:], bounce[:, :, :, :])
                else:
                    nc.scalar.copy(qtc[:], bounce[:, :, :, :])
                st[s]["qtc"][g] = qtc

            def emm(s, g):
                # symmetric Gram accumulation: upper-triangle blocks only,
                # packed E layout (3 PSUM banks)
                if "E" not in st[s]:
                    st[s]["E"] = ps_e.tile([128, 1280], F32, tag="E",
                                           name=f"E_{s}")
                E = st[s]["E"]
                qtc = st[s]["qtc"].pop(g)
                for h in range(2):
                    k = 2 * g + h
                    for m in range(CB):
                        # m=1 and m=3 share a PSUM bank; start=True clears
                        # has_written BANK-wide, so only m=1 may start the
                        # bank — m=3's first write lands on cleared bits
                        # (overwrite semantics) right after m=1's start
                        nc.tensor.matmul(
                            E[:, EOFF[m] : EOFF[m] + EW[m]],
                            lhsT=qtc[:, h, m * 128 : (m + 1) * 128],
                            rhs=qtc[:, h, m * 128 : 512],
                            start=(k == 0 and m != 3),
                            stop=(k == NK - 1),
                        )

            def esb_evac(s):
                # rebuild upper-tri rows in SBUF as fp16 (frees E PSUM)
                E = st[s]["E"]
                E_sb = esbp.tile([128, CB, 512], F16, tag="esb",
                                 name=f"esb_{s}")
                for m in range(CB):
                    nc.scalar.copy(E_sb[:, m, m * 128 : 512],
                                   E[:, EOFF[m] : EOFF[m] + EW[m]])
                st[s]["E_sb"] = E_sb

            def mirror_step(s, idx):
                # mirror one lower-triangle block via a fp16 PE transpose
                i, j = MIRROR_IJ[idx]
                E_sb = st[s]["E_sb"]
                tb = ps_o.tile([128, 128], F16, tag="acc",
                               name=f"tb_{s}_{i}_{j}")
                nc.tensor.transpose(
                    tb[:], E_sb[:, j, i * 128 : (i + 1) * 128], ident
                )
                nc.scalar.copy(E_sb[:, i, j * 128 : (j + 1) * 128], tb[:])

            def softmax_m(s, m):
                # A' = gamma/Z * exp(rowmin - E) via double exp pass;
                # then M = A' + I (diag add of the fp16 identity)
                E_sb = st[s]["E_sb"]
                if "expn" not in st[s]:
                    st[s]["expn"] = expnp.tile([128, CB, 512], F16,
                                               tag="expn", name=f"expn_{s}")
                expn = st[s]["expn"]
                mv = vecs.tile([128, 1], F32, tag="mv", name=f"mv_{s}_{m}")
                nc.vector.tensor_reduce(
                    mv, E_sb[:, m, :], axis=mybir.AxisListType.X,
                    op=mybir.AluOpType.min,
                )
                Z = vecs.tile([128, 1], F32, tag="Z", name=f"Z_{s}_{m}")
                nc.scalar.activation(
                    expn[:, m, :],
                    E_sb[:, m, :],
                    mybir.ActivationFunctionType.Exp,
                    bias=mv,
                    scale=-1.0,
                    accum_out=Z,
                )
                lnZ = vecs.tile([128, 1], F32, tag="lnZ", name=f"lnZ_{s}_{m}")
                nc.scalar.activation(lnZ, Z, mybir.ActivationFunctionType.Ln)
                mvg = vecs.tile([128, 1], F32, tag="mvg", name=f"mvg_{s}_{m}")
                nc.vector.tensor_add(mvg, mv, lng)
                b2 = vecs.tile([128, 1], F32, tag="b2", name=f"b2_{s}_{m}")
                nc.vector.tensor_sub(b2, mvg, lnZ)
                nc.scalar.activation(
                    expn[:, m, :],
                    E_sb[:, m, :],
                    mybir.ActivationFunctionType.Exp,
                    bias=b2,
                    scale=-1.0,
                )
                # M = A' + I on the diagonal block
                nc.vector.tensor_add(
                    expn[:, m, m * 128 : (m + 1) * 128],
                    expn[:, m, m * 128 : (m + 1) * 128],
                    ident[:],
                )

            def expT_step(s, j):
                expn = st[s]["expn"]
                bounce = ps_t.tile([128, CB, 128], F16, tag="bounce",
                                   name=f"ebounce_{s}_{j}")
                for cb in range(CB):
                    nc.tensor.transpose(
                        bounce[:, cb, :],
                        expn[:, cb, j * 128 : (j + 1) * 128],
                        ident,
                    )
                et = exptp.tile([128, CB, 128], F16, tag="expT",
                                name=f"expT_{s}_{j}")
                if j % 2 == 0:
                    nc.scalar.copy(et[:], bounce[:, :, :])
                else:
                    nc.vector.tensor_copy(et[:], bounce[:, :, :])
                st[s].setdefault("expT", {})[j] = et

            def aphase_cb(s, cb, ostage):
                # out = M @ q for one c-block, in no-PAIRS sharing each
                # weight load (2nd matmul sets ldweights=False); 3 acc
                # banks keep the pair pipeline free of evac stalls.
                # PSUM holds the final values; evac copies split DVE/ACT
                q16, expT = st[s]["q16"], st[s]["expT"]
                accs = {}
                for pair in range(NO // 2):
                    nos = (2 * pair, 2 * pair + 1)
                    for no in nos:
                        accs[no] = ps_o.tile([128, 512], F32, tag="acc",
                                             name=f"acc_{s}_{no}_{cb}")
                    for j in range(CB):
                        for idx, no in enumerate(nos):
                            npc, nof = _piece_of(no * 512)
                            mm = nc.tensor.matmul(
                                accs[no][:],
                                lhsT=expT[j][:, cb, :],
                                rhs=q16[j][npc][:, nof : nof + 512],
                                start=(j == 0),
                                stop=(j == CB - 1),
                            )
                            if idx == 1:
                                mm.ldweights = False
                    for idx, no in enumerate(nos):
                        half = no // (NO // 2)
                        if (cb, half) not in ostage:
                            ot = outsp.tile([128, (NO // 2) * 512], F32,
                                            tag="ot",
                                            name=f"ot_{s}_{cb}_{half}")
                            ostage[(cb, half)] = ot
                        ot = ostage[(cb, half)]
                        osl = slice((no % (NO // 2)) * 512,
                                    (no % (NO // 2) + 1) * 512)
                        if idx == 0:
                            nc.vector.tensor_copy(ot[:, osl], accs[no][:])
                        else:
                            nc.scalar.copy(ot[:, osl], accs[no][:])
                        _store(s, cb, no, ot)

            def _store(s, cb, no, ot):
                # the very last c-block stores in 512KB quarters so the
                # final DMA starts right after the last evac copy
                fine = (s == S - 1 and cb == CB - 1)
                if fine and no % 2 == 1:
                    q0 = (no - 1) % (NO // 2)
                    nc.sync.dma_start(
                        out=out_ap[
                            s * C + cb * 128 : s * C + (cb + 1) * 128,
                            (no - 1) * 512 : (no + 1) * 512,
                        ],
                        in_=ot[:, q0 * 512 : (q0 + 2) * 512],
                    )
                elif not fine and no % (NO // 2) == NO // 2 - 1:
                    half = no // (NO // 2)
                    nc.sync.dma_start(
                        out=out_ap[
                            s * C + cb * 128 : s * C + (cb + 1) * 128,
                            half * (NO // 2) * 512 :
                            (half + 1) * (NO // 2) * 512,
                        ],
                        in_=ot[:],
                    )

            # ---- interleaved emission schedule -----------------------
            # sample-0 loads up front, sample-1 load pieces interleaved
            # into the Gram(0) loop so DMA issue order matches arrival.
            # tgroups run TWO ahead of their emms: the bounce evacuation
            # of group g overlaps group g+1's transposes, so emm(g) never
            # exposes the evac latency on the PE queue
            for p in range(NP):
                load_piece(0, p)
            tgroup(0, 0)
            warm_fill(12)
            tgroup(0, 1)
            warm_fill(12)
            emm(0, 0)
            emm(0, 1)
            for g in range(2, NG):
                if g in (4, 8, 12):
                    # small transpose fills bridge DMA-arrival hiccups at
                    # load-piece boundaries so the HAM stays warm
                    warm_fill(4)
                tgroup(0, g)
                emm(0, g)
                if g == 8:
                    load_piece(1, 0)
                elif g == 10:
                    load_piece(1, 1)
                elif g == 12:
                    load_piece(1, 2)
                elif g == 14:
                    load_piece(1, 3)
            load_piece(1, 4)
            # E(0) PSUM is freed by esb_evac (all-ACT) so emm(1) can
            # start right after the evacuation copies
            esb_evac(0)
            tgroup(1, 0)
            tgroup(1, 1)
            emm(1, 0)
            mirror_step(0, 0)
            tgroup(1, 2); emm(1, 1)
            mirror_step(0, 1)
            tgroup(1, 3); emm(1, 2)
            mirror_step(0, 2)
            tgroup(1, 4); emm(1, 3)
            mirror_step(0, 3)
            tgroup(1, 5); emm(1, 4)
            mirror_step(0, 4)
            tgroup(1, 6); emm(1, 5)
            mirror_step(0, 5)
            for g in range(7, 11):
                tgroup(1, g)
                emm(1, g - 1)
            # softmax(0) in two chunks, early enough that the exp chain
            # completes before the PE reaches expT(0), late enough that
            # the remaining qtc evacuations stay ahead of it
            softmax_m(0, 0); softmax_m(0, 1)
            for g in range(11, 14):
                tgroup(1, g)
                emm(1, g - 1)
            softmax_m(0, 2); softmax_m(0, 3)
            tgroup(1, 14); emm(1, 13)
            tgroup(1, 15); emm(1, 14)
            emm(1, 15)
            for j in range(CB):
                expT_step(0, j)
            # aphase(0) with softmax(1) steps interleaved between c-blocks
            ostage0 = {}
            esb_evac(1)
            aphase_cb(0, 0, ostage0)
            mirror_step(1, 0); mirror_step(1, 1); mirror_step(1, 2)
            aphase_cb(0, 1, ostage0)
            mirror_step(1, 3); mirror_step(1, 4); mirror_step(1, 5)
            softmax_m(1, 0)
            aphase_cb(0, 2, ostage0)
            softmax_m(1, 1); softmax_m(1, 2); softmax_m(1, 3)
            aphase_cb(0, 3, ostage0)
            for j in range(CB):
                expT_step(1, j)
            ostage1 = {}
            for cb in range(CB):
                aphase_cb(1, cb, ostage1)
    return nc


def _split_excess_waits(nc, max_waits=1):
    """This container's walrus rejects >1 sync-wait on one instruction
    ("Too many sync wait commands"); hoist extras onto standalone
    InstEventSemaphore preludes on the same engine."""
    n = 0
    for fn in nc.m.functions:
        for bb in fn.blocks:
            out = []
            for inst in bb.instructions:
                si = inst.sync_info
                if si is not None and si.on_wait and len(si.on_wait) > max_waits:
                    waits = list(si.on_wait)
                    head, keep = waits[:-max_waits], waits[-max_waits:]
                    for i, w in enumerate(head):
                        ev = mybir.InstEventSemaphore(
                            name=f"{inst.name}-wsplit{i}", ins=[], outs=[])
                        ev.engine = inst.engine
                        ev.sync_info = mybir.SyncInfo(on_wait=[w], on_update=[])
                        out.append(ev)
                        n += 1
                    inst.sync_info = mybir.SyncInfo(
                        on_wait=keep, on_update=list(si.on_update))
                out.append(inst)
            bb.instructions[:] = out
    return n


_cache = {}


def _get_nc():
    if 'nc' not in _cache:
        nc = bass.Bass()
        build(nc)
        _split_excess_waits(nc)
        _cache['nc'] = nc
    return _cache['nc']


def kernel(x: np.ndarray, gamma: np.ndarray) -> np.ndarray:
    from concourse.bass_utils import run_bass_kernel_spmd

    B, CH, H, W = x.shape          # (16, 512, 64, 64)
    NSP = H * W
    M = 8                          # cores
    SS = B // M                    # samples per core
    nc = _get_nc()
    g = np.ascontiguousarray(gamma, dtype=np.float32).reshape(1, 1)
    in_maps = [
        {
            "x": np.ascontiguousarray(
                x[i * SS : (i + 1) * SS].reshape(SS * CH, NSP), dtype=np.float32
            ),
            "gamma": g,
        }
        for i in range(M)
    ]
    res = run_bass_kernel_spmd(nc, in_maps, core_ids=list(range(M)))
    out = np.concatenate(
        [res.results[i]["out"].reshape(SS, CH, H, W) for i in range(M)], axis=0
    )
    return np.ascontiguousarray(out, dtype=np.float32)


# revision 44
# speedup vs baseline: 1.0485x; 1.0187x over previous
"""Self-contained TRN2 Bass kernel for nn_CAM_Module (channel attention).

kernel(x, gamma): x [16,512,64,64] f32, gamma [1] f32 -> [16,512,64,64] f32.
Data-parallel over batch: 2 samples per NeuronCore across 8 cores.

Math: q = x.reshape(B,C,HW); E = q@q.T; softmax(rowmax(E)-E) == softmax(-E)
(shift invariance). Key folds:
  out = gamma*softmax(-E)@q + x = (gamma*softmax(-E) + I) @ q   since x == q
  gamma/Z scaling folded into the exp bias: A' = exp(rowmin + ln(gamma)
  - ln(Z) - E); M = A' + I; out = M @ q directly in PSUM -> epilogue is a
  plain PSUM->SBUF copy (split DVE/ACT) instead of scalar_tensor_tensor.

On-chip strategy (per core, 2 samples):
  - load fp32 in progressive pieces, cast to fp16 (DVE/ACT), PE-transpose
    128x128 tiles to build q^T chunks; single-pass fp16 Gram accumulated in
    fp32 PSUM (upper-triangle blocks only, packed into 3 PSUM banks),
    mirrored via fp16 PE transposes of the fp16 E_sb copy.
  - softmax: exp pass1 (fused rowsum via accum_out) -> ln(Z) on ACT ->
    exp pass2 with bias = rowmin + ln(gamma) - ln(Z) -> diag +1 add.
  - A-matmul fp16 with M^T tiles; PSUM holds the final output; evacuation
    copies alternate DVE/ACT into 1MB staging tiles.
  - PE pre-warmed with dummy transposes during the load ramp (HAM);
    softmax/mirror/expT steps interleave with Gram/A-phase emission so the
    PE never idles long enough to re-throttle.
"""
import sys
if '/opt/trn_rl_repo' not in sys.path:
    sys.path.insert(0, '/opt/trn_rl_repo')
import numpy as np
import concourse.bass as bass
import concourse.tile as tile
import concourse.mybir as mybir
from concourse.masks import make_identity

F32 = mybir.dt.float32
F16 = mybir.dt.float16

C = 512          # channels
N = 4096         # spatial (64*64)
CB = C // 128    # 4 c-blocks
NK = N // 128    # 32 transpose chunks
NG = NK // 2     # 16 transpose groups (2 chunks per PSUM bounce bank)
NO = N // 512    # 8 output column chunks
S = 2            # samples per core
WARM = 48        # dummy transposes to pre-warm the PE HAM clock gate

# load piece widths (columns) and offsets: finer first pieces cut the
# head latency; 512KB steady-state pieces pipeline the Gram phase
# against DMA arrival without starving the PE
P_W = [512, 512, 1024, 1024, 1024]
P_OFF = [0, 512, 1024, 2048, 3072]
NP = len(P_W)
# packed E PSUM layout: row-block m -> (offset, width); fits 3 banks,
# no block crosses a 2KB bank boundary (m3 placed before m2)
EW = [512, 384, 256, 128]
EOFF = [0, 512, 1024, 896]  # m0@0 m1@512 m3@896 m2@1024
MIRROR_IJ = [(1, 0), (2, 0), (2, 1), (3, 0), (3, 1), (3, 2)]


def _piece_of(n0):
    for p in range(NP):
        if P_OFF[p] <= n0 < P_OFF[p] + P_W[p]:
            return p, n0 - P_OFF[p]
    raise AssertionError(n0)


def build(nc: bass.Bass):
    x_ext = nc.declare_dram_parameter("x", [S * C, N], F32, isOutput=False)
    g_ext = nc.declare_dram_parameter("gamma", [1, 1], F32, isOutput=False)
    out_ext = nc.declare_dram_parameter("out", [S * C, N], F32, isOutput=True)
    x_ap = x_ext.ap()
    out_ap = out_ext.ap()

    with tile.TileContext(nc) as tc:
        with (
            tc.tile_pool(name="const", bufs=1) as const,
            tc.tile_pool(name="x32", bufs=3) as x32,
            tc.tile_pool(name="q16", bufs=S * CB) as q16p,
            tc.tile_pool(name="qt", bufs=6) as qtp,
            tc.tile_pool(name="esb", bufs=2) as esbp,
            tc.tile_pool(name="expn", bufs=2) as expnp,
            tc.tile_pool(name="expt", bufs=2 * CB) as exptp,
            tc.tile_pool(name="vecs", bufs=8) as vecs,
            tc.tile_pool(name="outs", bufs=3) as outsp,
            tc.tile_pool(name="ps_bounce", bufs=2, space="PSUM") as ps_t,
            tc.tile_pool(name="ps_e", bufs=1, space="PSUM") as ps_e,
            tc.tile_pool(name="ps_o", bufs=3, space="PSUM") as ps_o,
        ):
            # PE pre-warm: back-to-back dummy matmuls on a DVE-memset
            # scratch flip the HAM clock gate to 8/8 before the real
            # stream begins -- no dependency on the (slow, gpsimd-built)
            # identity, so the PE starts almost immediately
            scratch = const.tile([128, 128], F16)
            nc.vector.memset(scratch, 0.0)
            warm = ps_t.tile([128, 512], F32, tag="bounce", name="warm")
            for _ in range(WARM):
                nc.tensor.matmul(warm[:, 0:128], lhsT=scratch[:],
                                 rhs=scratch[:], start=True, stop=True)

            ident = const.tile([128, 128], F16)
            make_identity(nc, ident)
            gbc = const.tile([128, 1], F32)
            nc.gpsimd.dma_start(out=gbc, in_=g_ext.ap().to_broadcast((128, 1)))
            lng = const.tile([128, 1], F32)
            nc.scalar.activation(lng, gbc, mybir.ActivationFunctionType.Ln)

            st = [dict() for _ in range(S)]

            def warm_fill(n):
                # keep the PE stream dense during the load ramp (HAM)
                for _ in range(n):
                    nc.tensor.matmul(warm[:, 0:128], lhsT=scratch[:],
                                     rhs=scratch[:], start=True, stop=True)

            def load_piece(s, p):
                # casts all on DVE (2x mode for fp32-src copies; ACT gets
                # no accel on fp32 sources and was the v2 bottleneck)
                if "q16" not in st[s]:
                    st[s]["q16"] = [[None] * NP for _ in range(CB)]
                    st[s]["qtc"] = {}
                q16 = st[s]["q16"]
                for cb in range(CB):
                    xt = x32.tile([128, P_W[p]], F32, tag=f"xt{p}",
                                  name=f"xt_{s}_{cb}_{p}")
                    nc.sync.dma_start(
                        out=xt,
                        in_=x_ap[
                            s * C + cb * 128 : s * C + (cb + 1) * 128,
                            P_OFF[p] : P_OFF[p] + P_W[p],
                        ],
                    )
                    qc = q16p.tile([128, P_W[p]], F16, tag=f"q16_{p}",
                                   name=f"q16_{s}_{cb}_{p}")
                    nc.vector.tensor_copy(qc[:], xt[:])
                    q16[cb][p] = qc

            def tgroup(s, g):
                # transpose 2 chunks (8 [128,128] fp16 tiles) into one
                # PSUM bounce bank, evacuate to SBUF in one op
                q16 = st[s]["q16"]
                bounce = ps_t.tile([128, 2, CB, 128], F16, tag="bounce",
                                   name=f"bounce_{s}_{g}")
                for h in range(2):
                    k = 2 * g + h
                    kp, ko = _piece_of(128 * k)
                    for cb in range(CB):
                        nc.tensor.transpose(
                            bounce[:, h, cb, :],
                            q16[cb][kp][:, ko : ko + 128],
                            ident,
                        )
                qtc = qtp.tile([128, 2, CB * 128], F16, tag="qtc",
                               name=f"qtc_{s}_{g}")
                # DVE reads fp16 PSUM at 2x (650ns), ACT only 1x (1.1us);
                # alternate so neither queue gates the bounce-bank ring
                if g % 2 == 0:
                    nc.vector.tensor_copy(qtc[:], bounce[:, :, :, :])
                else:
                    nc.scalar.copy(qtc[:], bounce[:, :, :, :])
                st[s]["qtc"][g] = qtc

            def emm(s, g):
                # symmetric Gram accumulation: upper-triangle blocks only,
                # packed E layout (3 PSUM banks)
                if "E" not in st[s]:
                    st[s]["E"] = ps_e.tile([128, 1280], F32, tag="E",
                                           name=f"E_{s}")
                E = st[s]["E"]
                qtc = st[s]["qtc"].pop(g)
                for h in range(2):
                    k = 2 * g + h
                    for m in range(CB):
                        # m=1 and m=3 share a PSUM bank; start=True clears
                        # has_written BANK-wide, so only m=1 may start the
                        # bank — m=3's first write lands on cleared bits
                        # (overwrite semantics) right after m=1's start
                        nc.tensor.matmul(
                            E[:, EOFF[m] : EOFF[m] + EW[m]],
                            lhsT=qtc[:, h, m * 128 : (m + 1) * 128],
                            rhs=qtc[:, h, m * 128 : 512],
                            start=(k == 0 and m != 3),
                            stop=(k == NK - 1),
                        )

            def esb_evac(s):
                # rebuild upper-tri rows in SBUF as fp16 (frees E PSUM)
                E = st[s]["E"]
                E_sb = esbp.tile([128, CB, 512], F16, tag="esb",
                                 name=f"esb_{s}")
                for m in range(CB):
                    nc.scalar.copy(E_sb[:, m, m * 128 : 512],
                                   E[:, EOFF[m] : EOFF[m] + EW[m]])
                st[s]["E_sb"] = E_sb

            def mirror_step(s, idx):
                # mirror one lower-triangle block via a fp16 PE transpose
                i, j = MIRROR_IJ[idx]
                E_sb = st[s]["E_sb"]
                tb = ps_o.tile([128, 128], F16, tag="acc",
                               name=f"tb_{s}_{i}_{j}")
                nc.tensor.transpose(
                    tb[:], E_sb[:, j, i * 128 : (i + 1) * 128], ident
                )
                nc.scalar.copy(E_sb[:, i, j * 128 : (j + 1) * 128], tb[:])

            def softmax_m(s, m):
                # A' = gamma/Z * exp(rowmin - E) via double exp pass;
                # then M = A' + I (diag add of the fp16 identity)
                E_sb = st[s]["E_sb"]
                if "expn" not in st[s]:
                    st[s]["expn"] = expnp.tile([128, CB, 512], F16,
                                               tag="expn", name=f"expn_{s}")
                expn = st[s]["expn"]
                mv = vecs.tile([128, 1], F32, tag="mv", name=f"mv_{s}_{m}")
                nc.vector.tensor_reduce(
                    mv, E_sb[:, m, :], axis=mybir.AxisListType.X,
                    op=mybir.AluOpType.min,
                )
                Z = vecs.tile([128, 1], F32, tag="Z", name=f"Z_{s}_{m}")
                nc.scalar.activation(
                    expn[:, m, :],
                    E_sb[:, m, :],
                    mybir.ActivationFunctionType.Exp,
                    bias=mv,
                    scale=-1.0,
                    accum_out=Z,
                )
                lnZ = vecs.tile([128, 1], F32, tag="lnZ", name=f"lnZ_{s}_{m}")
                nc.scalar.activation(lnZ, Z, mybir.ActivationFunctionType.Ln)
                mvg = vecs.tile([128, 1], F32, tag="mvg", name=f"mvg_{s}_{m}")
                nc.vector.tensor_add(mvg, mv, lng)
                b2 = vecs.tile([128, 1], F32, tag="b2", name=f"b2_{s}_{m}")
                nc.vector.tensor_sub(b2, mvg, lnZ)
                nc.scalar.activation(
                    expn[:, m, :],
                    E_sb[:, m, :],
                    mybir.ActivationFunctionType.Exp,
                    bias=b2,
                    scale=-1.0,
                )
                # M = A' + I on the diagonal block
                nc.vector.tensor_add(
                    expn[:, m, m * 128 : (m + 1) * 128],
                    expn[:, m, m * 128 : (m + 1) * 128],
                    ident[:],
                )

            def expT_step(s, j):
                expn = st[s]["expn"]
                bounce = ps_t.tile([128, CB, 128], F16, tag="bounce",
                                   name=f"ebounce_{s}_{j}")
                for cb in range(CB):
                    nc.tensor.transpose(
                        bounce[:, cb, :],
                        expn[:, cb, j * 128 : (j + 1) * 128],
                        ident,
                    )
                et = exptp.tile([128, CB, 128], F16, tag="expT",
                                name=f"expT_{s}_{j}")
                if j % 2 == 0:
                    nc.scalar.copy(et[:], bounce[:, :, :])
                else:
                    nc.vector.tensor_copy(et[:], bounce[:, :, :])
                st[s].setdefault("expT", {})[j] = et

            def aphase_cb(s, cb, ostage):
                # out = M @ q for one c-block, in no-PAIRS sharing each
                # weight load (2nd matmul sets ldweights=False); 3 acc
                # banks keep the pair pipeline free of evac stalls.
                # PSUM holds the final values; evac copies split DVE/ACT
                q16, expT = st[s]["q16"], st[s]["expT"]
                accs = {}
                for pair in range(NO // 2):
                    nos = (2 * pair, 2 * pair + 1)
                    for no in nos:
                        accs[no] = ps_o.tile([128, 512], F32, tag="acc",
                                             name=f"acc_{s}_{no}_{cb}")
                    for j in range(CB):
                        for idx, no in enumerate(nos):
                            npc, nof = _piece_of(no * 512)
                            mm = nc.tensor.matmul(
                                accs[no][:],
                                lhsT=expT[j][:, cb, :],
                                rhs=q16[j][npc][:, nof : nof + 512],
                                start=(j == 0),
                                stop=(j == CB - 1),
                            )
                            if idx == 1:
                                mm.ldweights = False
                    for idx, no in enumerate(nos):
                        half = no // (NO // 2)
                        if (cb, half) not in ostage:
                            ot = outsp.tile([128, (NO // 2) * 512], F32,
                                            tag="ot",
                                            name=f"ot_{s}_{cb}_{half}")
                            ostage[(cb, half)] = ot
                        ot = ostage[(cb, half)]
                        osl = slice((no % (NO // 2)) * 512,
                                    (no % (NO // 2) + 1) * 512)
                        if idx == 0:
                            nc.vector.tensor_copy(ot[:, osl], accs[no][:])
                        else:
                            nc.scalar.copy(ot[:, osl], accs[no][:])
                        _store(s, cb, no, ot)

            def _store(s, cb, no, ot):
                # the very last c-block stores in 512KB quarters so the
                # final DMA starts right after the last evac copy
                fine = (s == S - 1 and cb == CB - 1)
                if fine and no % 2 == 1:
                    q0 = (no - 1) % (NO // 2)
                    nc.sync.dma_start(
                        out=out_ap[
                            s * C + cb * 128 : s * C + (cb + 1) * 128,
                            (no - 1) * 512 : (no + 1) * 512,
                        ],
                        in_=ot[:, q0 * 512 : (q0 + 2) * 512],
                    )
                elif not fine and no % (NO // 2) == NO // 2 - 1:
                    half = no // (NO // 2)
                    nc.sync.dma_start(
                        out=out_ap[
                            s * C + cb * 128 : s * C + (cb + 1) * 128,
                            half * (NO // 2) * 512 :
                            (half + 1) * (NO // 2) * 512,
                        ],
                        in_=ot[:],
                    )

            # ---- interleaved emission schedule -----------------------
            # sample-0 loads up front, sample-1 load pieces interleaved
            # into the Gram(0) loop so DMA issue order matches arrival.
            # tgroups run TWO ahead of their emms: the bounce evacuation
            # of group g overlaps group g+1's transposes, so emm(g) never
            # exposes the evac latency on the PE queue
            for p in range(NP):
                load_piece(0, p)
            tgroup(0, 0)
            warm_fill(12)
            tgroup(0, 1)
            warm_fill(12)
            emm(0, 0)
            emm(0, 1)
            for g in range(2, NG):
                if g in (3, 4, 8, 12):
                    # small matmul fills bridge DMA-arrival hiccups at
                    # load-piece boundaries so the HAM stays warm
                    warm_fill(6)
                tgroup(0, g)
                emm(0, g)
                if g == 8:
                    load_piece(1, 0)
                elif g == 10:
                    load_piece(1, 1)
                elif g == 12:
                    load_piece(1, 2)
                elif g == 14:
                    load_piece(1, 3)
            load_piece(1, 4)
            # E(0) PSUM is freed by esb_evac (all-ACT) so emm(1) can
            # start right after the evacuation copies
            esb_evac(0)
            tgroup(1, 0)
            tgroup(1, 1)
            emm(1, 0)
            mirror_step(0, 0)
            tgroup(1, 2); emm(1, 1)
            mirror_step(0, 1)
            tgroup(1, 3); emm(1, 2)
            mirror_step(0, 2)
            tgroup(1, 4); emm(1, 3)
            mirror_step(0, 3)
            tgroup(1, 5); emm(1, 4)
            mirror_step(0, 4)
            tgroup(1, 6); emm(1, 5)
            mirror_step(0, 5)
            for g in range(7, 11):
                tgroup(1, g)
                emm(1, g - 1)
            # softmax(0) in two chunks, early enough that the exp chain
            # completes before the PE reaches expT(0), late enough that
            # the remaining qtc evacuations stay ahead of it
            softmax_m(0, 0); softmax_m(0, 1)
            tgroup(1, 11); emm(1, 10)
            softmax_m(0, 2)
            tgroup(1, 12); emm(1, 11)
            softmax_m(0, 3)
            tgroup(1, 13); emm(1, 12)
            tgroup(1, 14); emm(1, 13)
            tgroup(1, 15); emm(1, 14)
            emm(1, 15)
            # bridge the softmax(0) -> expT(0) handoff so the HAM never
            # re-throttles before the A-phase
            warm_fill(16)
            for j in range(CB):
                expT_step(0, j)
            # aphase(0) with softmax(1) steps interleaved between c-blocks
            ostage0 = {}
            esb_evac(1)
            aphase_cb(0, 0, ostage0)
            mirror_step(1, 0); mirror_step(1, 1); mirror_step(1, 2)
            aphase_cb(0, 1, ostage0)
            mirror_step(1, 3); mirror_step(1, 4); mirror_step(1, 5)
            softmax_m(1, 0)
            aphase_cb(0, 2, ostage0)
            softmax_m(1, 1); softmax_m(1, 2); softmax_m(1, 3)
            aphase_cb(0, 3, ostage0)
            warm_fill(8)
            for j in range(CB):
                expT_step(1, j)
            ostage1 = {}
            for cb in range(CB):
                aphase_cb(1, cb, ostage1)
    return nc


def _split_excess_waits(nc, max_waits=1):
    """This container's walrus rejects >1 sync-wait on one instruction
    ("Too many sync wait commands"); hoist extras onto standalone
    InstEventSemaphore preludes on the same engine."""
    n = 0
    for fn in nc.m.functions:
        for bb in fn.blocks:
            out = []
            for inst in bb.instructions:
                si = inst.sync_info
                if si is not None and si.on_wait and len(si.on_wait) > max_waits:
                    waits = list(si.on_wait)
                    head, keep = waits[:-max_waits], waits[-max_waits:]
                    for i, w in enumerate(head):
                        ev = mybir.InstEventSemaphore(
                            name=f"{inst.name}-wsplit{i}", ins=[], outs=[])
                        ev.engine = inst.engine
                        ev.sync_info = mybir.SyncInfo(on_wait=[w], on_update=[])
                        out.append(ev)
                        n += 1
                    inst.sync_info = mybir.SyncInfo(
                        on_wait=keep, on_update=list(si.on_update))
                out.append(inst)
            bb.instructions[:] = out
    return n


_cache = {}


def _get_nc():
    if 'nc' not in _cache:
        nc = bass.Bass()
        build(nc)
        _split_excess_waits(nc)
        _cache['nc'] = nc
    return _cache['nc']


def kernel(x: np.ndarray, gamma: np.ndarray) -> np.ndarray:
    from concourse.bass_utils import run_bass_kernel_spmd

    B, CH, H, W = x.shape          # (16, 512, 64, 64)
    NSP = H * W
    M = 8                          # cores
    SS = B // M                    # samples per core
    nc = _get_nc()
    g = np.ascontiguousarray(gamma, dtype=np.float32).reshape(1, 1)
    in_maps = [
        {
            "x": np.ascontiguousarray(
                x[i * SS : (i + 1) * SS].reshape(SS * CH, NSP), dtype=np.float32
            ),
            "gamma": g,
        }
        for i in range(M)
    ]
    res = run_bass_kernel_spmd(nc, in_maps, core_ids=list(range(M)))
    out = np.concatenate(
        [res.results[i]["out"].reshape(SS, CH, H, W) for i in range(M)], axis=0
    )
    return np.ascontiguousarray(out, dtype=np.float32)
